# revision 3
# baseline (speedup 1.0000x reference)
"""Trainium2 Bass kernel for nn_DQNAgent (topk_masking).

kernel(**inputs) takes the FULL unsharded inputs from setup_inputs() and
returns (state, actions) exactly like reference().  Pure data-parallel over
8 NeuronCores (batch 4096 -> 512/core), weights replicated; one NEFF runs
the feature MLP + the 100-step selection loop per core.

All matmuls native fp32 (PE 4 cyc/row).  Sigmoid computed as
0.5*(1+tanh(x/2)) via the ACT Tanh table (4-ULP) with the 0.5 factors
folded exactly (power-of-two scalings) into host-side weight copies:
  stored h~ = 2*h, S = 2*c  =>  Whh/2, Wb/2, Ws/2, 2*W_fe2.
"""

import numpy as np

B = 4096
NCORE = 8
BL = B // NCORE          # 512 batch per core
NQ = BL // 128           # 4 partition tiles of 128
N_BLOCK = 100
N_SLOT = 300
N_YS = 1236
N_STATE = 1436
HID = 256
H4 = 4 * HID
STEPS = 100
CHUNK = 64               # MLP batch chunk (per core)
NCHUNK = BL // CHUNK

_CACHE = {}


def _build(steps=STEPS):
    import concourse.bass as bass
    import concourse.mybir as mybir
    import concourse.tile as tile
    from concourse import bacc
    from concourse.masks import make_identity

    dt = mybir.dt
    Alu = mybir.AluOpType
    AF = mybir.ActivationFunctionType
    AX = mybir.AxisListType
    ds = bass.ds

    nc = bacc.Bacc(None, target_bir_lowering=False)

    def din(name, shape):
        return nc.dram_tensor(name, list(shape), dt.float32, kind="ExternalInput")

    # ---- DRAM inputs (per core) ----
    to_r = din("to_r", (BL, N_BLOCK * 5))          # to_tensor rows, original layout
    ti_r = din("ti_r", (BL, N_BLOCK * 5))
    ys_r = din("ys_r", (BL, N_YS * 5))
    to_feats_d = din("to_feats", (BL, 5 * N_BLOCK))  # to.transpose(0,2,1) feature-major
    d0_d = din("d0", (BL, N_SLOT))
    d1_d = din("d1", (BL, N_SLOT))
    cnt_d = din("cnt", (BL, N_SLOT))
    madd_to_d = din("madd_to", (BL, N_BLOCK))
    madd_ti_d = din("madd_ti", (BL, N_BLOCK))
    w1t_d = din("w1t", (N_STATE, 512))             # W1.T
    w2t_d = din("w2t", (512, 512))
    w3t_d = din("w3t", (512, 256))
    b1_d = din("b1p", (128, 4))
    b2_d = din("b2p", (128, 4))
    b3_d = din("b3p", (128, 2))
    w2f_d = din("w2f", (128, 5))                   # 2*W_fe2 replicated
    bfe_d = din("bfe", (128, 1))                   # 2*b_fe2 replicated
    whht_to_d = din("whht_to", (HID, H4))          # (Whh_to/2).T
    whht_ti_d = din("whht_ti", (HID, H4))
    wiht_to_d = din("wiht_to", (5, H4))            # Wih_to.T
    bg_to_d = din("bg_to", (128, 8))               # gate bias cols (halved for i,f,o)
    bg_ti_d = din("bg_ti", (128, 8))
    wbt_d = din("wbt", (HID, N_BLOCK))             # (Wb/2).T
    wst_d = din("wst", (HID, N_SLOT))              # (Ws/2).T
    bs_d = din("bs_row", (1, N_SLOT))

    act_b_d = nc.dram_tensor("act_b", [BL, steps], dt.int32, kind="ExternalOutput")
    act_s_d = nc.dram_tensor("act_s", [BL, steps], dt.int32, kind="ExternalOutput")
    act_t_d = nc.dram_tensor("act_t", [BL, steps], dt.int32, kind="ExternalOutput")

    KT1 = [(k * 128, min(128, N_STATE - k * 128)) for k in range(12)]  # layer1 k tiles

    with tile.TileContext(nc) as tc:
        with (
            tc.tile_pool(name="wp", bufs=1) as wp,        # weights / constants
            tc.tile_pool(name="st", bufs=1) as st,        # persistent state
        ):
            # ---- load weights & constants ----
            w1t = [wp.tile([kn, 512], dt.float32, tag=f"w1t{k}", name=f"w1t{k}")
                   for k, (k0, kn) in enumerate(KT1)]
            for k, (k0, kn) in enumerate(KT1):
                nc.sync.dma_start(w1t[k][:], w1t_d[k0:k0 + kn, :])
            w2t = [wp.tile([128, 512], dt.float32, tag=f"w2t{k}", name=f"w2t{k}")
                   for k in range(4)]
            for k in range(4):
                nc.sync.dma_start(w2t[k][:], w2t_d[k * 128:(k + 1) * 128, :])
            w3t = [wp.tile([128, 256], dt.float32, tag=f"w3t{k}", name=f"w3t{k}")
                   for k in range(4)]
            for k in range(4):
                nc.sync.dma_start(w3t[k][:], w3t_d[k * 128:(k + 1) * 128, :])
            b1p = wp.tile([128, 4], dt.float32)
            b2p = wp.tile([128, 4], dt.float32)
            b3p = wp.tile([128, 2], dt.float32)
            w2f = wp.tile([128, 5], dt.float32)
            bfe = wp.tile([128, 1], dt.float32)
            for t_, d_ in ((b1p, b1_d), (b2p, b2_d), (b3p, b3_d), (w2f, w2f_d), (bfe, bfe_d)):
                nc.sync.dma_start(t_[:], d_[:])
            whht_to = [wp.tile([128, H4], dt.float32, tag=f"whhto{k}", name=f"whhto{k}")
                       for k in range(2)]
            whht_ti = [wp.tile([128, H4], dt.float32, tag=f"whhti{k}", name=f"whhti{k}")
                       for k in range(2)]
            for k in range(2):
                nc.sync.dma_start(whht_to[k][:], whht_to_d[k * 128:(k + 1) * 128, :])
                nc.sync.dma_start(whht_ti[k][:], whht_ti_d[k * 128:(k + 1) * 128, :])
            wiht_to = wp.tile([5, H4], dt.float32)
            nc.sync.dma_start(wiht_to[:], wiht_to_d[:])
            bg_to = wp.tile([128, 8], dt.float32)
            bg_ti = wp.tile([128, 8], dt.float32)
            nc.sync.dma_start(bg_to[:], bg_to_d[:])
            nc.sync.dma_start(bg_ti[:], bg_ti_d[:])
            wbt = [wp.tile([128, N_BLOCK], dt.float32, tag=f"wbt{k}", name=f"wbt{k}")
                   for k in range(2)]
            wst = [wp.tile([128, N_SLOT], dt.float32, tag=f"wst{k}", name=f"wst{k}")
                   for k in range(2)]
            for k in range(2):
                nc.sync.dma_start(wbt[k][:], wbt_d[k * 128:(k + 1) * 128, :])
                nc.sync.dma_start(wst[k][:], wst_d[k * 128:(k + 1) * 128, :])
            bs_row = wp.tile([1, N_SLOT], dt.float32)
            nc.sync.dma_start(bs_row[:], bs_d[:])
            ones1 = wp.tile([1, 128], dt.float32)
            nc.vector.memset(ones1[:], 1.0)
            ident = wp.tile([128, 128], dt.float32)
            make_identity(nc, ident[:])
            iota_i = wp.tile([128, N_SLOT], dt.int32)
            nc.gpsimd.iota(iota_i[:], pattern=[[1, N_SLOT]], base=0, channel_multiplier=0)
            iota_f = wp.tile([128, N_SLOT], dt.float32)
            nc.vector.tensor_copy(iota_f[:], iota_i[:])

            # ---- persistent per-q state ----
            to_feats = [st.tile([128, 5 * N_BLOCK], dt.float32, tag=f"tof{q}", name=f"tof{q}")
                        for q in range(NQ)]
            d0p = [st.tile([128, N_SLOT], dt.float32, tag=f"d0{q}", name=f"d0{q}")
                   for q in range(NQ)]
            d1p = [st.tile([128, N_SLOT], dt.float32, tag=f"d1{q}", name=f"d1{q}")
                   for q in range(NQ)]
            cntp = [st.tile([128, N_SLOT], dt.float32, tag=f"cnt{q}", name=f"cnt{q}")
                    for q in range(NQ)]
            madd_to = [st.tile([128, N_BLOCK], dt.float32, tag=f"mato{q}", name=f"mato{q}")
                       for q in range(NQ)]
            madd_ti = [st.tile([128, N_BLOCK], dt.float32, tag=f"mati{q}", name=f"mati{q}")
                       for q in range(NQ)]
            actb = [st.tile([128, steps], dt.int32, tag=f"actb{q}", name=f"actb{q}")
                    for q in range(NQ)]
            acts_ = [st.tile([128, steps], dt.int32, tag=f"acts{q}", name=f"acts{q}")
                     for q in range(NQ)]
            actt = [st.tile([128, steps], dt.int32, tag=f"actt{q}", name=f"actt{q}")
                    for q in range(NQ)]
            for q in range(NQ):
                r0 = q * 128
                nc.sync.dma_start(to_feats[q][:], to_feats_d[r0:r0 + 128, :])
                nc.sync.dma_start(d0p[q][:], d0_d[r0:r0 + 128, :])
                nc.sync.dma_start(d1p[q][:], d1_d[r0:r0 + 128, :])
                nc.sync.dma_start(cntp[q][:], cnt_d[r0:r0 + 128, :])
                nc.sync.dma_start(madd_to[q][:], madd_to_d[r0:r0 + 128, :])
                nc.sync.dma_start(madd_ti[q][:], madd_ti_d[r0:r0 + 128, :])

            # LSTM state (transposed layout [H,B]); h~ = 2h, S = 2c
            hto = [st.tile([128, BL], dt.float32, tag=f"hto{k}", name=f"hto{k}") for k in range(2)]
            hti = [st.tile([128, BL], dt.float32, tag=f"hti{k}", name=f"hti{k}") for k in range(2)]
            Sto = [st.tile([128, BL], dt.float32, tag=f"Sto{k}", name=f"Sto{k}") for k in range(2)]
            Sti = [st.tile([128, BL], dt.float32, tag=f"Sti{k}", name=f"Sti{k}") for k in range(2)]
            for k in range(2):
                nc.vector.memset(Sto[k][:], 0.0)
                nc.vector.memset(Sti[k][:], 0.0)
            xT = st.tile([5, BL], dt.float32)
            nc.vector.memset(xT[:], 0.0)

            # =================== Phase 1: feature MLP ===================
            with (
                tc.tile_pool(name="mlp", bufs=2) as mp,
                tc.tile_pool(name="mlq", bufs=3) as mq,
                tc.tile_pool(name="mps", bufs=1, space="PSUM") as mps,
            ):
                for ch in range(NCHUNK):
                    b0 = ch * CHUNK
                    stch = mp.tile([CHUNK, N_STATE, 5], dt.float32, tag="stch", bufs=1,
                                   name="stch")
                    nc.sync.dma_start(stch[:, 0:N_BLOCK, :],
                                      to_r[b0:b0 + CHUNK, :].rearrange("b (n j) -> b n j", j=5))
                    nc.sync.dma_start(stch[:, N_BLOCK:2 * N_BLOCK, :],
                                      ti_r[b0:b0 + CHUNK, :].rearrange("b (n j) -> b n j", j=5))
                    nc.sync.dma_start(stch[:, 2 * N_BLOCK:N_STATE, :],
                                      ys_r[b0:b0 + CHUNK, :].rearrange("b (n j) -> b n j", j=5))

                    ps1 = [mps.tile([128, CHUNK * 5], dt.float32, tag="mlpacc", bufs=6,
                                    name=f"ps1_{m}") for m in range(4)]
                    for ki, (k0, kn) in enumerate(KT1):
                        xt = mq.tile([kn, CHUNK * 5], dt.float32, tag="xt", name="xt")
                        for j in range(5):
                            ptr = mps.tile([kn, CHUNK], dt.float32, tag="ptr", bufs=2,
                                           name="ptr")
                            nc.tensor.matmul(ptr[:], stch[:, k0:k0 + kn, j],
                                             ident[0:CHUNK, 0:CHUNK], start=True, stop=True)
                            nc.vector.tensor_copy(
                                xt[:].rearrange("k (b j) -> k b j", j=5)[:, :, j], ptr[:])
                        for m in range(4):
                            nc.tensor.matmul(ps1[m][:], w1t[ki][:, m * 128:(m + 1) * 128],
                                             xt[:], start=(ki == 0), stop=(ki == 11))
                    o1 = [mp.tile([128, CHUNK * 5], dt.float32, tag=f"o1_{m}", name=f"o1_{m}")
                          for m in range(4)]
                    for m in range(4):
                        nc.scalar.activation(o1[m][:], ps1[m][:], AF.Relu,
                                             bias=b1p[:, m:m + 1], scale=1.0)
                    ps2 = [mps.tile([128, CHUNK * 5], dt.float32, tag="mlpacc", bufs=6,
                                    name=f"ps2_{m}") for m in range(4)]
                    for k in range(4):
                        for m in range(4):
                            nc.tensor.matmul(ps2[m][:], w2t[k][:, m * 128:(m + 1) * 128],
                                             o1[k][:], start=(k == 0), stop=(k == 3))
                    o2 = [mp.tile([128, CHUNK * 5], dt.float32, tag=f"o2_{m}", name=f"o2_{m}")
                          for m in range(4)]
                    for m in range(4):
                        nc.scalar.activation(o2[m][:], ps2[m][:], AF.Relu,
                                             bias=b2p[:, m:m + 1], scale=1.0)
                    ps3 = [mps.tile([128, CHUNK * 5], dt.float32, tag="mlpacc", bufs=6,
                                    name=f"ps3_{m}") for m in range(2)]
                    for k in range(4):
                        for m in range(2):
                            nc.tensor.matmul(ps3[m][:], w3t[k][:, m * 128:(m + 1) * 128],
                                             o2[k][:], start=(k == 0), stop=(k == 3))
                    o3 = [mp.tile([128, CHUNK * 5], dt.float32, tag=f"o3_{m}", name=f"o3_{m}")
                          for m in range(2)]
                    for m in range(2):
                        nc.scalar.activation(o3[m][:], ps3[m][:], AF.Relu,
                                             bias=b3p[:, m:m + 1], scale=1.0)
                    # feat~ = 2*feat = sum_j o3[:, b, j]*(2*W_fe2)[j] + 2*b_fe2
                    for m in range(2):
                        ftmp = mp.tile([128, CHUNK, 5], dt.float32, tag="ftmp", name="ftmp")
                        nc.vector.tensor_tensor(
                            ftmp[:], o3[m][:].rearrange("p (b j) -> p b j", j=5),
                            w2f[:].unsqueeze(1).broadcast_to([128, CHUNK, 5]), Alu.mult)
                        fred = mp.tile([128, CHUNK], dt.float32, tag="fred", name="fred")
                        nc.vector.tensor_reduce(fred[:], ftmp[:], AX.X, Alu.add)
                        fh0 = mp.tile([128, CHUNK], dt.float32, tag="fh0", name="fh0")
                        nc.vector.tensor_scalar(fh0[:], fred[:], bfe[:, 0:1], None, Alu.add)
                        nc.vector.tensor_copy(hto[m][:, b0:b0 + CHUNK], fh0[:])
                        nc.vector.tensor_copy(hti[m][:, b0:b0 + CHUNK], fh0[:])

            # =================== Phase 2: the 100-step loop ===================
            GSCALE = [0.5, 0.5, 0.5, 0.5, 1.0, 1.0, 0.5, 0.5]  # i,i,f,f,g,g,o,o

            with (
                tc.tile_pool(name="sm", bufs=2) as sm,
                tc.tile_pool(name="gt", bufs=1) as gt,
                tc.tile_pool(name="lps", bufs=1, space="PSUM") as lps,
            ):
                with tc.For_i(0, steps) as t:
                    # ---- to-LSTM ----
                    tg_to = {}
                    for m in range(8):
                        pg = lps.tile([128, BL], dt.float32, tag="psg", bufs=3, name="pg")
                        ms = slice(m * 128, (m + 1) * 128)
                        nc.tensor.matmul(pg[:], whht_to[0][:, ms], hto[0][:], start=True, stop=False)
                        nc.tensor.matmul(pg[:], whht_to[1][:, ms], hto[1][:], start=False, stop=False)
                        nc.tensor.matmul(pg[:], wiht_to[:, ms], xT[:], start=False, stop=True)
                        tt = gt.tile([128, BL], dt.float32, tag=f"tgto{m}", name=f"tgto{m}")
                        nc.scalar.activation(tt[:], pg[:], AF.Tanh,
                                             bias=bg_to[:, m:m + 1], scale=GSCALE[m])
                        tg_to[m] = tt
                    for h in range(2):
                        u = sm.tile([128, BL], dt.float32, tag="u", bufs=1, name="u")
                        v = sm.tile([128, BL], dt.float32, tag="v", bufs=1, name="v")
                        nc.vector.scalar_tensor_tensor(u[:], tg_to[2 + h][:], 1.0, Sto[h][:],
                                                       Alu.add, Alu.mult)
                        nc.vector.scalar_tensor_tensor(v[:], tg_to[0 + h][:], 1.0, tg_to[4 + h][:],
                                                       Alu.add, Alu.mult)
                        nc.vector.scalar_tensor_tensor(Sto[h][:], u[:], 0.5, v[:],
                                                       Alu.mult, Alu.add)
                        tch = sm.tile([128, BL], dt.float32, tag="tch", bufs=1, name="tch")
                        nc.scalar.activation(tch[:], Sto[h][:], AF.Tanh, bias=0.0, scale=0.5)
                        nc.vector.scalar_tensor_tensor(hto[h][:], tg_to[6 + h][:], 1.0, tch[:],
                                                       Alu.add, Alu.mult)
                    # ---- ti-LSTM (zero input) ----
                    tg_ti = {}
                    for m in range(8):
                        pg = lps.tile([128, BL], dt.float32, tag="psg", bufs=3, name="pg")
                        ms = slice(m * 128, (m + 1) * 128)
                        nc.tensor.matmul(pg[:], whht_ti[0][:, ms], hti[0][:], start=True, stop=False)
                        nc.tensor.matmul(pg[:], whht_ti[1][:, ms], hti[1][:], start=False, stop=True)
                        tt = gt.tile([128, BL], dt.float32, tag=f"tgti{m}", name=f"tgti{m}")
                        nc.scalar.activation(tt[:], pg[:], AF.Tanh,
                                             bias=bg_ti[:, m:m + 1], scale=GSCALE[m])
                        tg_ti[m] = tt
                    for h in range(2):
                        u = sm.tile([128, BL], dt.float32, tag="u", bufs=1, name="u")
                        v = sm.tile([128, BL], dt.float32, tag="v", bufs=1, name="v")
                        nc.vector.scalar_tensor_tensor(u[:], tg_ti[2 + h][:], 1.0, Sti[h][:],
                                                       Alu.add, Alu.mult)
                        nc.vector.scalar_tensor_tensor(v[:], tg_ti[0 + h][:], 1.0, tg_ti[4 + h][:],
                                                       Alu.add, Alu.mult)
                        nc.vector.scalar_tensor_tensor(Sti[h][:], u[:], 0.5, v[:],
                                                       Alu.mult, Alu.add)
                        tch = sm.tile([128, BL], dt.float32, tag="tch", bufs=1, name="tch")
                        nc.scalar.activation(tch[:], Sti[h][:], AF.Tanh, bias=0.0, scale=0.5)
                        nc.vector.scalar_tensor_tensor(hti[h][:], tg_ti[6 + h][:], 1.0, tch[:],
                                                       Alu.add, Alu.mult)

                    for q in range(NQ):
                        qs = slice(q * 128, (q + 1) * 128)
                        # ---- block selection (to) ----
                        plb = lps.tile([128, N_BLOCK], dt.float32, tag="pslog", bufs=2,
                                       name="plb")
                        nc.tensor.matmul(plb[:], hto[0][:, qs], wbt[0][:], start=True, stop=False)
                        nc.tensor.matmul(plb[:], hto[1][:, qs], wbt[1][:], start=False, stop=True)
                        mb = sm.tile([128, N_BLOCK], dt.float32, tag="mb", name="mb")
                        nc.vector.tensor_tensor(mb[:], plb[:], madd_to[q][:], Alu.add)
                        mxb = sm.tile([128, 1], dt.float32, tag="mxb", name="mxb")
                        nc.vector.tensor_reduce(mxb[:], mb[:], AX.X, Alu.max)
                        idxb = sm.tile([128, 8], dt.uint32, tag="idxb", name="idxb")
                        nc.vector.max_index(idxb[:], mxb[:, 0:1].broadcast_to([128, 8]), mb[:])
                        nc.vector.tensor_copy(actb[q][:, ds(t, 1)], idxb[:, 0:1])
                        selb = sm.tile([128, 1], dt.float32, tag="selb", name="selb")
                        nc.vector.tensor_copy(selb[:], idxb[:, 0:1])
                        ohb = sm.tile([128, N_BLOCK], dt.float32, tag="ohb", name="ohb")
                        nc.vector.tensor_scalar(ohb[:], iota_f[:, 0:N_BLOCK], selb[:, 0:1],
                                                None, Alu.is_equal)
                        nc.vector.scalar_tensor_tensor(madd_to[q][:], ohb[:], -2000.0,
                                                       madd_to[q][:], Alu.mult, Alu.add)
                        gtmp = sm.tile([128, 5, N_BLOCK], dt.float32, tag="gtmp", name="gtmp")
                        nc.vector.tensor_tensor(
                            gtmp[:], to_feats[q][:].rearrange("p (j n) -> p j n", j=5),
                            ohb[:].unsqueeze(1).broadcast_to([128, 5, N_BLOCK]), Alu.mult)
                        inpq = sm.tile([128, 5], dt.float32, tag="inpq", name="inpq")
                        nc.vector.tensor_reduce(inpq[:], gtmp[:], AX.X, Alu.add)
                        pxt = lps.tile([5, 128], dt.float32, tag="psxt", bufs=2, name="pxt")
                        nc.tensor.matmul(pxt[:], inpq[:], ident[:], start=True, stop=True)
                        nc.vector.tensor_copy(xT[:, qs], pxt[:])
                        # ---- slot selection ----
                        pls = lps.tile([128, N_SLOT], dt.float32, tag="pslog", bufs=2,
                                       name="pls")
                        nc.tensor.matmul(pls[:], hto[0][:, qs], wst[0][:], start=True, stop=False)
                        nc.tensor.matmul(pls[:], hto[1][:, qs], wst[1][:], start=False, stop=False)
                        nc.tensor.matmul(pls[:], ones1[:, 0:128], bs_row[:], start=False, stop=True)
                        ge0 = sm.tile([128, N_SLOT], dt.float32, tag="ge0", name="ge0")
                        ge1 = sm.tile([128, N_SLOT], dt.float32, tag="ge1", name="ge1")
                        nc.gpsimd.tensor_scalar(ge0[:], d0p[q][:], inpq[:, 0:1], None, Alu.is_gt)
                        nc.gpsimd.tensor_scalar(ge1[:], d1p[q][:], inpq[:, 1:2], None, Alu.is_gt)
                        fe01 = sm.tile([128, N_SLOT], dt.float32, tag="fe01", name="fe01")
                        nc.gpsimd.tensor_tensor(fe01[:], ge0[:], ge1[:], Alu.mult)
                        alv = sm.tile([128, N_SLOT], dt.float32, tag="alv", name="alv")
                        nc.gpsimd.tensor_scalar(alv[:], cntp[q][:], 0.0, None, Alu.is_gt)
                        feas = sm.tile([128, N_SLOT], dt.float32, tag="feas", name="feas")
                        nc.gpsimd.tensor_tensor(feas[:], fe01[:], alv[:], Alu.mult)
                        tadd = sm.tile([128, N_SLOT], dt.float32, tag="tadd", name="tadd")
                        nc.gpsimd.tensor_scalar(tadd[:], feas[:], 2000.0, -2000.0,
                                                Alu.mult, Alu.add)
                        msk = sm.tile([128, N_SLOT], dt.float32, tag="msk", name="msk")
                        nc.vector.tensor_tensor(msk[:], pls[:], tadd[:], Alu.add)
                        mxs = sm.tile([128, 1], dt.float32, tag="mxs", name="mxs")
                        nc.vector.tensor_reduce(mxs[:], msk[:], AX.X, Alu.max)
                        idxs = sm.tile([128, 8], dt.uint32, tag="idxs", name="idxs")
                        nc.vector.max_index(idxs[:], mxs[:, 0:1].broadcast_to([128, 8]), msk[:])
                        nc.vector.tensor_copy(acts_[q][:, ds(t, 1)], idxs[:, 0:1])
                        sels = sm.tile([128, 1], dt.float32, tag="sels", name="sels")
                        nc.vector.tensor_copy(sels[:], idxs[:, 0:1])
                        ohs = sm.tile([128, N_SLOT], dt.float32, tag="ohs", name="ohs")
                        nc.vector.tensor_scalar(ohs[:], iota_f[:], sels[:, 0:1], None,
                                                Alu.is_equal)
                        nc.vector.scalar_tensor_tensor(cntp[q][:], ohs[:], -1.0, cntp[q][:],
                                                       Alu.mult, Alu.add)
                        # ---- block selection (ti) ----
                        plt = lps.tile([128, N_BLOCK], dt.float32, tag="pslog", bufs=2,
                                       name="plt")
                        nc.tensor.matmul(plt[:], hti[0][:, qs], wbt[0][:], start=True, stop=False)
                        nc.tensor.matmul(plt[:], hti[1][:, qs], wbt[1][:], start=False, stop=True)
                        mt = sm.tile([128, N_BLOCK], dt.float32, tag="mt", name="mt")
                        nc.vector.tensor_tensor(mt[:], plt[:], madd_ti[q][:], Alu.add)
                        mxt = sm.tile([128, 1], dt.float32, tag="mxt", name="mxt")
                        nc.vector.tensor_reduce(mxt[:], mt[:], AX.X, Alu.max)
                        idxt = sm.tile([128, 8], dt.uint32, tag="idxt", name="idxt")
                        nc.vector.max_index(idxt[:], mxt[:, 0:1].broadcast_to([128, 8]), mt[:])
                        nc.vector.tensor_copy(actt[q][:, ds(t, 1)], idxt[:, 0:1])
                        selt = sm.tile([128, 1], dt.float32, tag="selt", name="selt")
                        nc.vector.tensor_copy(selt[:], idxt[:, 0:1])
                        oht = sm.tile([128, N_BLOCK], dt.float32, tag="oht", name="oht")
                        nc.vector.tensor_scalar(oht[:], iota_f[:, 0:N_BLOCK], selt[:, 0:1],
                                                None, Alu.is_equal)
                        nc.vector.scalar_tensor_tensor(madd_ti[q][:], oht[:], -2000.0,
                                                       madd_ti[q][:], Alu.mult, Alu.add)

                for q in range(NQ):
                    r0 = q * 128
                    nc.sync.dma_start(act_b_d[r0:r0 + 128, :], actb[q][:])
                    nc.sync.dma_start(act_s_d[r0:r0 + 128, :], acts_[q][:])
                    nc.sync.dma_start(act_t_d[r0:r0 + 128, :], actt[q][:])

    nc.compile()
    return nc


def _prep_inputs(inputs):
    """Host-side exact preprocessing -> per-core in_maps."""
    f32 = np.float32
    to = np.ascontiguousarray(inputs["to_tensor"], dtype=f32)
    ti = np.ascontiguousarray(inputs["ti_tensor"], dtype=f32)
    ys = np.ascontiguousarray(inputs["ys_tensor"], dtype=f32)
    slot = np.ascontiguousarray(inputs["slot_info"], dtype=f32)
    W1 = np.asarray(inputs["W1"], f32); b1 = np.asarray(inputs["b1"], f32)
    W2 = np.asarray(inputs["W2"], f32); b2 = np.asarray(inputs["b2"], f32)
    W3 = np.asarray(inputs["W3"], f32); b3 = np.asarray(inputs["b3"], f32)
    W_fe2 = np.asarray(inputs["W_fe2"], f32); b_fe2 = np.asarray(inputs["b_fe2"], f32)
    Wih_to = np.asarray(inputs["Wih_to"], f32); Whh_to = np.asarray(inputs["Whh_to"], f32)
    bih_to = np.asarray(inputs["bih_to"], f32); bhh_to = np.asarray(inputs["bhh_to"], f32)
    Whh_ti = np.asarray(inputs["Whh_ti"], f32)
    bih_ti = np.asarray(inputs["bih_ti"], f32); bhh_ti = np.asarray(inputs["bhh_ti"], f32)
    Wb = np.asarray(inputs["Wb"], f32); bb = np.asarray(inputs["bb"], f32)
    Ws = np.asarray(inputs["Ws"], f32); bs = np.asarray(inputs["bs"], f32)

    def gate_bias_pack(bsum):
        cols = []
        for m in range(8):
            seg = bsum[m * 128:(m + 1) * 128]
            cols.append(seg if m in (4, 5) else seg * 0.5)
        return np.stack(cols, axis=1).astype(f32)

    shared = {
        "w1t": np.ascontiguousarray(W1.T),
        "w2t": np.ascontiguousarray(W2.T),
        "w3t": np.ascontiguousarray(W3.T),
        "b1p": np.ascontiguousarray(b1.reshape(4, 128).T),
        "b2p": np.ascontiguousarray(b2.reshape(4, 128).T),
        "b3p": np.ascontiguousarray(b3.reshape(2, 128).T),
        "w2f": np.broadcast_to(2.0 * W_fe2[0], (128, 5)).astype(f32),
        "bfe": np.full((128, 1), 2.0 * b_fe2[0], f32),
        "whht_to": np.ascontiguousarray((Whh_to * 0.5).T),
        "whht_ti": np.ascontiguousarray((Whh_ti * 0.5).T),
        "wiht_to": np.ascontiguousarray(Wih_to.T),
        "bg_to": gate_bias_pack((bih_to + bhh_to).astype(f32)),
        "bg_ti": gate_bias_pack((bih_ti + bhh_ti).astype(f32)),
        "wbt": np.ascontiguousarray((Wb * 0.5).T),
        "wst": np.ascontiguousarray((Ws * 0.5).T),
        "bs_row": bs.reshape(1, N_SLOT),
    }
    in_maps = []
    for c in range(NCORE):
        s = slice(c * BL, (c + 1) * BL)
        toc, tic, ysc, slc = to[s], ti[s], ys[s], slot[s]
        m_to = np.where(toc[:, :, 0] != -1.0, bb[None, :], bb[None, :] - 2000.0).astype(f32)
        m_ti = np.where(tic[:, :, 0] != -1.0, bb[None, :], bb[None, :] - 2000.0).astype(f32)
        in_maps.append(dict(shared,
            to_r=np.ascontiguousarray(toc.reshape(BL, -1)),
            ti_r=np.ascontiguousarray(tic.reshape(BL, -1)),
            ys_r=np.ascontiguousarray(ysc.reshape(BL, -1)),
            to_feats=np.ascontiguousarray(toc.transpose(0, 2, 1)).reshape(BL, -1),
            d0=np.ascontiguousarray(slc[:, :, 0]),
            d1=np.ascontiguousarray(slc[:, :, 1]),
            cnt=np.ascontiguousarray(slc[:, :, 4]),
            madd_to=m_to, madd_ti=m_ti))
    return in_maps


def kernel(**inputs):
    if "nc" not in _CACHE:
        _CACHE["nc"] = _build(STEPS)
    nc = _CACHE["nc"]
    from concourse.bass_utils import run_bass_kernel_spmd
    in_maps = _prep_inputs(inputs)
    results = run_bass_kernel_spmd(nc, in_maps, list(range(NCORE))).results
    actions = np.empty((B, STEPS, 3), np.int32)
    for c in range(NCORE):
        s = slice(c * BL, (c + 1) * BL)
        actions[s, :, 0] = results[c]["act_b"]
        actions[s, :, 1] = results[c]["act_s"]
        actions[s, :, 2] = results[c]["act_t"]
    state = np.concatenate([
        np.asarray(inputs["to_tensor"], np.float32),
        np.asarray(inputs["ti_tensor"], np.float32),
        np.asarray(inputs["ys_tensor"], np.float32)], axis=1)
    return state, actions


# revision 4
# speedup vs baseline: 390.9833x; 390.9833x over previous
"""Trainium2 Bass kernel for nn_DQNAgent (topk_masking).

kernel(**inputs) takes the FULL unsharded inputs from setup_inputs() and
returns (state, actions) exactly like reference().  Pure data-parallel over
8 NeuronCores (batch 4096 -> 512/core), weights replicated; one NEFF runs
the feature MLP + the 100-step selection loop per core.

All matmuls native fp32 (PE 4 cyc/row).  Sigmoid computed as
0.5*(1+tanh(x/2)) via the ACT Tanh table (4-ULP) with the 0.5 factors
folded exactly (power-of-two scalings) into host-side weight copies:
  stored h~ = 2*h, S = 2*c  =>  Whh/2, Wb/2, Ws/2, 2*W_fe2.
"""

import numpy as np

B = 4096
NCORE = 8
BL = B // NCORE          # 512 batch per core
NQ = BL // 128           # 4 partition tiles of 128
N_BLOCK = 100
N_SLOT = 300
N_YS = 1236
N_STATE = 1436
HID = 256
H4 = 4 * HID
STEPS = 100
CHUNK = 64               # MLP batch chunk (per core)
NCHUNK = BL // CHUNK

_CACHE = {}


def _build(steps=STEPS):
    import concourse.bass as bass
    import concourse.mybir as mybir
    import concourse.tile as tile
    from concourse import bacc
    from concourse.masks import make_identity

    dt = mybir.dt
    Alu = mybir.AluOpType
    AF = mybir.ActivationFunctionType
    AX = mybir.AxisListType
    ds = bass.ds

    nc = bacc.Bacc(None, target_bir_lowering=False)

    def din(name, shape):
        return nc.dram_tensor(name, list(shape), dt.float32, kind="ExternalInput")

    # ---- DRAM inputs (per core) ----
    to_r = din("to_r", (BL, N_BLOCK * 5))          # to_tensor rows, original layout
    ti_r = din("ti_r", (BL, N_BLOCK * 5))
    ys_r = din("ys_r", (BL, N_YS * 5))
    to_feats_d = din("to_feats", (BL, 5 * N_BLOCK))  # to.transpose(0,2,1) feature-major
    d0_d = din("d0", (BL, N_SLOT))
    d1_d = din("d1", (BL, N_SLOT))
    cnt_d = din("cnt", (BL, N_SLOT))
    madd_to_d = din("madd_to", (BL, N_BLOCK))
    madd_ti_d = din("madd_ti", (BL, N_BLOCK))
    w1t_d = din("w1t", (N_STATE, 512))             # W1.T
    w2t_d = din("w2t", (512, 512))
    w3t_d = din("w3t", (512, 256))
    b1_d = din("b1p", (128, 4))
    b2_d = din("b2p", (128, 4))
    b3_d = din("b3p", (128, 2))
    w2f_d = din("w2f", (128, 5))                   # 2*W_fe2 replicated
    bfe_d = din("bfe", (128, 1))                   # 2*b_fe2 replicated
    whht_to_d = din("whht_to", (HID, H4))          # (Whh_to/2).T
    whht_ti_d = din("whht_ti", (HID, H4))
    wiht_to_d = din("wiht_to", (5, H4))            # Wih_to.T
    bg_to_d = din("bg_to", (128, 8))               # gate bias cols (halved for i,f,o)
    bg_ti_d = din("bg_ti", (128, 8))
    wbt_d = din("wbt", (HID, N_BLOCK))             # (Wb/2).T
    wst_d = din("wst", (HID, N_SLOT))              # (Ws/2).T
    bs_d = din("bs_row", (1, N_SLOT))

    act_b_d = nc.dram_tensor("act_b", [BL, steps], dt.int32, kind="ExternalOutput")
    act_s_d = nc.dram_tensor("act_s", [BL, steps], dt.int32, kind="ExternalOutput")
    act_t_d = nc.dram_tensor("act_t", [BL, steps], dt.int32, kind="ExternalOutput")

    KT1 = [(k * 128, min(128, N_STATE - k * 128)) for k in range(12)]  # layer1 k tiles

    with tile.TileContext(nc) as tc:
        with (
            tc.tile_pool(name="wp", bufs=1) as wp,        # weights / constants
            tc.tile_pool(name="st", bufs=1) as st,        # persistent state
        ):
            # ---- load weights & constants ----
            w1t = [wp.tile([kn, 512], dt.float32, tag=f"w1t{k}", name=f"w1t{k}")
                   for k, (k0, kn) in enumerate(KT1)]
            for k, (k0, kn) in enumerate(KT1):
                nc.sync.dma_start(w1t[k][:], w1t_d[k0:k0 + kn, :])
            w2t = [wp.tile([128, 512], dt.float32, tag=f"w2t{k}", name=f"w2t{k}")
                   for k in range(4)]
            for k in range(4):
                nc.sync.dma_start(w2t[k][:], w2t_d[k * 128:(k + 1) * 128, :])
            w3t = [wp.tile([128, 256], dt.float32, tag=f"w3t{k}", name=f"w3t{k}")
                   for k in range(4)]
            for k in range(4):
                nc.sync.dma_start(w3t[k][:], w3t_d[k * 128:(k + 1) * 128, :])
            b1p = wp.tile([128, 4], dt.float32)
            b2p = wp.tile([128, 4], dt.float32)
            b3p = wp.tile([128, 2], dt.float32)
            w2f = wp.tile([128, 5], dt.float32)
            bfe = wp.tile([128, 1], dt.float32)
            for t_, d_ in ((b1p, b1_d), (b2p, b2_d), (b3p, b3_d), (w2f, w2f_d), (bfe, bfe_d)):
                nc.sync.dma_start(t_[:], d_[:])
            whht_to = [wp.tile([128, H4], dt.float32, tag=f"whhto{k}", name=f"whhto{k}")
                       for k in range(2)]
            whht_ti = [wp.tile([128, H4], dt.float32, tag=f"whhti{k}", name=f"whhti{k}")
                       for k in range(2)]
            for k in range(2):
                nc.sync.dma_start(whht_to[k][:], whht_to_d[k * 128:(k + 1) * 128, :])
                nc.sync.dma_start(whht_ti[k][:], whht_ti_d[k * 128:(k + 1) * 128, :])
            wiht_to = wp.tile([5, H4], dt.float32)
            nc.sync.dma_start(wiht_to[:], wiht_to_d[:])
            bg_to = wp.tile([128, 8], dt.float32)
            bg_ti = wp.tile([128, 8], dt.float32)
            nc.sync.dma_start(bg_to[:], bg_to_d[:])
            nc.sync.dma_start(bg_ti[:], bg_ti_d[:])
            wbt = [wp.tile([128, N_BLOCK], dt.float32, tag=f"wbt{k}", name=f"wbt{k}")
                   for k in range(2)]
            wst = [wp.tile([128, N_SLOT], dt.float32, tag=f"wst{k}", name=f"wst{k}")
                   for k in range(2)]
            for k in range(2):
                nc.sync.dma_start(wbt[k][:], wbt_d[k * 128:(k + 1) * 128, :])
                nc.sync.dma_start(wst[k][:], wst_d[k * 128:(k + 1) * 128, :])
            bs_row = wp.tile([1, N_SLOT], dt.float32)
            nc.sync.dma_start(bs_row[:], bs_d[:])
            ones1 = wp.tile([1, 128], dt.float32)
            nc.vector.memset(ones1[:], 1.0)
            ident = wp.tile([128, 128], dt.float32)
            make_identity(nc, ident[:])
            iota_i = wp.tile([128, N_SLOT], dt.int32)
            nc.gpsimd.iota(iota_i[:], pattern=[[1, N_SLOT]], base=0, channel_multiplier=0)
            iota_f = wp.tile([128, N_SLOT], dt.float32)
            nc.vector.tensor_copy(iota_f[:], iota_i[:])

            # ---- persistent per-q state ----
            to_feats = [st.tile([128, 5 * N_BLOCK], dt.float32, tag=f"tof{q}", name=f"tof{q}")
                        for q in range(NQ)]
            d0p = [st.tile([128, N_SLOT], dt.float32, tag=f"d0{q}", name=f"d0{q}")
                   for q in range(NQ)]
            d1p = [st.tile([128, N_SLOT], dt.float32, tag=f"d1{q}", name=f"d1{q}")
                   for q in range(NQ)]
            cntp = [st.tile([128, N_SLOT], dt.float32, tag=f"cnt{q}", name=f"cnt{q}")
                    for q in range(NQ)]
            madd_to = [st.tile([128, N_BLOCK], dt.float32, tag=f"mato{q}", name=f"mato{q}")
                       for q in range(NQ)]
            madd_ti = [st.tile([128, N_BLOCK], dt.float32, tag=f"mati{q}", name=f"mati{q}")
                       for q in range(NQ)]
            actb = [st.tile([128, steps], dt.int32, tag=f"actb{q}", name=f"actb{q}")
                    for q in range(NQ)]
            acts_ = [st.tile([128, steps], dt.int32, tag=f"acts{q}", name=f"acts{q}")
                     for q in range(NQ)]
            actt = [st.tile([128, steps], dt.int32, tag=f"actt{q}", name=f"actt{q}")
                    for q in range(NQ)]
            for q in range(NQ):
                r0 = q * 128
                nc.sync.dma_start(to_feats[q][:], to_feats_d[r0:r0 + 128, :])
                nc.sync.dma_start(d0p[q][:], d0_d[r0:r0 + 128, :])
                nc.sync.dma_start(d1p[q][:], d1_d[r0:r0 + 128, :])
                nc.sync.dma_start(cntp[q][:], cnt_d[r0:r0 + 128, :])
                nc.sync.dma_start(madd_to[q][:], madd_to_d[r0:r0 + 128, :])
                nc.sync.dma_start(madd_ti[q][:], madd_ti_d[r0:r0 + 128, :])

            # LSTM state (transposed layout [H,B]); h~ = 2h, S = 2c
            hto = [st.tile([128, BL], dt.float32, tag=f"hto{k}", name=f"hto{k}") for k in range(2)]
            hti = [st.tile([128, BL], dt.float32, tag=f"hti{k}", name=f"hti{k}") for k in range(2)]
            Sto = [st.tile([128, BL], dt.float32, tag=f"Sto{k}", name=f"Sto{k}") for k in range(2)]
            Sti = [st.tile([128, BL], dt.float32, tag=f"Sti{k}", name=f"Sti{k}") for k in range(2)]
            for k in range(2):
                nc.vector.memset(Sto[k][:], 0.0)
                nc.vector.memset(Sti[k][:], 0.0)
            xT = st.tile([5, BL], dt.float32)
            nc.vector.memset(xT[:], 0.0)

            # =================== Phase 1: feature MLP ===================
            with (
                tc.tile_pool(name="mlp", bufs=2) as mp,
                tc.tile_pool(name="mlq", bufs=3) as mq,
                tc.tile_pool(name="mps", bufs=1, space="PSUM") as mps,
            ):
                for ch in range(NCHUNK):
                    b0 = ch * CHUNK
                    stch = mp.tile([CHUNK, N_STATE, 5], dt.float32, tag="stch", bufs=1,
                                   name="stch")
                    nc.sync.dma_start(stch[:, 0:N_BLOCK, :],
                                      to_r[b0:b0 + CHUNK, :].rearrange("b (n j) -> b n j", j=5))
                    nc.sync.dma_start(stch[:, N_BLOCK:2 * N_BLOCK, :],
                                      ti_r[b0:b0 + CHUNK, :].rearrange("b (n j) -> b n j", j=5))
                    nc.sync.dma_start(stch[:, 2 * N_BLOCK:N_STATE, :],
                                      ys_r[b0:b0 + CHUNK, :].rearrange("b (n j) -> b n j", j=5))

                    ps1 = [mps.tile([128, CHUNK * 5], dt.float32, tag="mlpacc", bufs=6,
                                    name=f"ps1_{m}") for m in range(4)]
                    for ki, (k0, kn) in enumerate(KT1):
                        xt = mq.tile([kn, CHUNK * 5], dt.float32, tag="xt", name="xt")
                        for j in range(5):
                            ptr = mps.tile([kn, CHUNK], dt.float32, tag="ptr", bufs=2,
                                           name="ptr")
                            nc.tensor.matmul(ptr[:], stch[:, k0:k0 + kn, j],
                                             ident[0:CHUNK, 0:CHUNK], start=True, stop=True)
                            nc.vector.tensor_copy(
                                xt[:].rearrange("k (b j) -> k b j", j=5)[:, :, j], ptr[:])
                        for m in range(4):
                            nc.tensor.matmul(ps1[m][:], w1t[ki][:, m * 128:(m + 1) * 128],
                                             xt[:], start=(ki == 0), stop=(ki == 11))
                    o1 = [mp.tile([128, CHUNK * 5], dt.float32, tag=f"o1_{m}", name=f"o1_{m}")
                          for m in range(4)]
                    for m in range(4):
                        nc.scalar.activation(o1[m][:], ps1[m][:], AF.Relu,
                                             bias=b1p[:, m:m + 1], scale=1.0)
                    ps2 = [mps.tile([128, CHUNK * 5], dt.float32, tag="mlpacc", bufs=6,
                                    name=f"ps2_{m}") for m in range(4)]
                    for k in range(4):
                        for m in range(4):
                            nc.tensor.matmul(ps2[m][:], w2t[k][:, m * 128:(m + 1) * 128],
                                             o1[k][:], start=(k == 0), stop=(k == 3))
                    o2 = [mp.tile([128, CHUNK * 5], dt.float32, tag=f"o2_{m}", name=f"o2_{m}")
                          for m in range(4)]
                    for m in range(4):
                        nc.scalar.activation(o2[m][:], ps2[m][:], AF.Relu,
                                             bias=b2p[:, m:m + 1], scale=1.0)
                    ps3 = [mps.tile([128, CHUNK * 5], dt.float32, tag="mlpacc", bufs=6,
                                    name=f"ps3_{m}") for m in range(2)]
                    for k in range(4):
                        for m in range(2):
                            nc.tensor.matmul(ps3[m][:], w3t[k][:, m * 128:(m + 1) * 128],
                                             o2[k][:], start=(k == 0), stop=(k == 3))
                    o3 = [mp.tile([128, CHUNK * 5], dt.float32, tag=f"o3_{m}", name=f"o3_{m}")
                          for m in range(2)]
                    for m in range(2):
                        nc.scalar.activation(o3[m][:], ps3[m][:], AF.Relu,
                                             bias=b3p[:, m:m + 1], scale=1.0)
                    # feat~ = 2*feat = sum_j o3[:, b, j]*(2*W_fe2)[j] + 2*b_fe2
                    for m in range(2):
                        ftmp = mp.tile([128, CHUNK, 5], dt.float32, tag="ftmp", name="ftmp")
                        nc.vector.tensor_tensor(
                            ftmp[:], o3[m][:].rearrange("p (b j) -> p b j", j=5),
                            w2f[:].unsqueeze(1).broadcast_to([128, CHUNK, 5]), Alu.mult)
                        fred = mp.tile([128, CHUNK], dt.float32, tag="fred", name="fred")
                        nc.vector.tensor_reduce(fred[:], ftmp[:], AX.X, Alu.add)
                        fh0 = mp.tile([128, CHUNK], dt.float32, tag="fh0", name="fh0")
                        nc.vector.tensor_scalar(fh0[:], fred[:], bfe[:, 0:1], None, Alu.add)
                        nc.vector.tensor_copy(hto[m][:, b0:b0 + CHUNK], fh0[:])
                        nc.vector.tensor_copy(hti[m][:, b0:b0 + CHUNK], fh0[:])

            # =================== Phase 2: the 100-step loop ===================
            GSCALE = [0.5, 0.5, 0.5, 0.5, 1.0, 1.0, 0.5, 0.5]  # i,i,f,f,g,g,o,o

            with (
                tc.tile_pool(name="sm", bufs=2) as sm,
                tc.tile_pool(name="gt", bufs=1) as gt,
                tc.tile_pool(name="lps", bufs=1, space="PSUM") as lps,
            ):
                with tc.For_i(0, steps) as t:
                    # ---- to-LSTM ----
                    tg_to = {}
                    for m in range(8):
                        pg = lps.tile([128, BL], dt.float32, tag="psg", bufs=3, name="pg")
                        ms = slice(m * 128, (m + 1) * 128)
                        nc.tensor.matmul(pg[:], whht_to[0][:, ms], hto[0][:], start=True, stop=False)
                        nc.tensor.matmul(pg[:], whht_to[1][:, ms], hto[1][:], start=False, stop=False)
                        nc.tensor.matmul(pg[:], wiht_to[:, ms], xT[:], start=False, stop=True)
                        tt = gt.tile([128, BL], dt.float32, tag=f"tgto{m}", name=f"tgto{m}")
                        nc.scalar.activation(tt[:], pg[:], AF.Tanh,
                                             bias=bg_to[:, m:m + 1], scale=GSCALE[m])
                        tg_to[m] = tt
                    for h in range(2):
                        u = sm.tile([128, BL], dt.float32, tag="u", bufs=1, name="u")
                        v = sm.tile([128, BL], dt.float32, tag="v", bufs=1, name="v")
                        nc.vector.scalar_tensor_tensor(u[:], tg_to[2 + h][:], 1.0, Sto[h][:],
                                                       Alu.add, Alu.mult)
                        nc.vector.scalar_tensor_tensor(v[:], tg_to[0 + h][:], 1.0, tg_to[4 + h][:],
                                                       Alu.add, Alu.mult)
                        nc.vector.scalar_tensor_tensor(Sto[h][:], u[:], 0.5, v[:],
                                                       Alu.mult, Alu.add)
                        tch = sm.tile([128, BL], dt.float32, tag="tch", bufs=1, name="tch")
                        nc.scalar.activation(tch[:], Sto[h][:], AF.Tanh, bias=0.0, scale=0.5)
                        nc.vector.scalar_tensor_tensor(hto[h][:], tg_to[6 + h][:], 1.0, tch[:],
                                                       Alu.add, Alu.mult)
                    # ---- ti-LSTM (zero input) ----
                    tg_ti = {}
                    for m in range(8):
                        pg = lps.tile([128, BL], dt.float32, tag="psg", bufs=3, name="pg")
                        ms = slice(m * 128, (m + 1) * 128)
                        nc.tensor.matmul(pg[:], whht_ti[0][:, ms], hti[0][:], start=True, stop=False)
                        nc.tensor.matmul(pg[:], whht_ti[1][:, ms], hti[1][:], start=False, stop=True)
                        tt = gt.tile([128, BL], dt.float32, tag=f"tgti{m}", name=f"tgti{m}")
                        nc.scalar.activation(tt[:], pg[:], AF.Tanh,
                                             bias=bg_ti[:, m:m + 1], scale=GSCALE[m])
                        tg_ti[m] = tt
                    for h in range(2):
                        u = sm.tile([128, BL], dt.float32, tag="u", bufs=1, name="u")
                        v = sm.tile([128, BL], dt.float32, tag="v", bufs=1, name="v")
                        nc.vector.scalar_tensor_tensor(u[:], tg_ti[2 + h][:], 1.0, Sti[h][:],
                                                       Alu.add, Alu.mult)
                        nc.vector.scalar_tensor_tensor(v[:], tg_ti[0 + h][:], 1.0, tg_ti[4 + h][:],
                                                       Alu.add, Alu.mult)
                        nc.vector.scalar_tensor_tensor(Sti[h][:], u[:], 0.5, v[:],
                                                       Alu.mult, Alu.add)
                        tch = sm.tile([128, BL], dt.float32, tag="tch", bufs=1, name="tch")
                        nc.scalar.activation(tch[:], Sti[h][:], AF.Tanh, bias=0.0, scale=0.5)
                        nc.vector.scalar_tensor_tensor(hti[h][:], tg_ti[6 + h][:], 1.0, tch[:],
                                                       Alu.add, Alu.mult)

                    for q in range(NQ):
                        qs = slice(q * 128, (q + 1) * 128)
                        # ---- block selection (to) ----
                        plb = lps.tile([128, N_BLOCK], dt.float32, tag="pslog", bufs=2,
                                       name="plb")
                        nc.tensor.matmul(plb[:], hto[0][:, qs], wbt[0][:], start=True, stop=False)
                        nc.tensor.matmul(plb[:], hto[1][:, qs], wbt[1][:], start=False, stop=True)
                        mb = sm.tile([128, N_BLOCK], dt.float32, tag="mb", name="mb")
                        nc.vector.tensor_tensor(mb[:], plb[:], madd_to[q][:], Alu.add)
                        mxb = sm.tile([128, 1], dt.float32, tag="mxb", name="mxb")
                        nc.vector.tensor_reduce(mxb[:], mb[:], AX.X, Alu.max)
                        idxb = sm.tile([128, 8], dt.uint32, tag="idxb", name="idxb")
                        nc.vector.max_index(idxb[:], mxb[:, 0:1].broadcast_to([128, 8]), mb[:])
                        nc.vector.tensor_copy(actb[q][:, ds(t, 1)], idxb[:, 0:1])
                        selb = sm.tile([128, 1], dt.float32, tag="selb", name="selb")
                        nc.vector.tensor_copy(selb[:], idxb[:, 0:1])
                        ohb = sm.tile([128, N_BLOCK], dt.float32, tag="ohb", name="ohb")
                        nc.vector.tensor_scalar(ohb[:], iota_f[:, 0:N_BLOCK], selb[:, 0:1],
                                                None, Alu.is_equal)
                        nc.vector.scalar_tensor_tensor(madd_to[q][:], ohb[:], -2000.0,
                                                       madd_to[q][:], Alu.mult, Alu.add)
                        gtmp = sm.tile([128, 5, N_BLOCK], dt.float32, tag="gtmp", name="gtmp")
                        nc.vector.tensor_tensor(
                            gtmp[:], to_feats[q][:].rearrange("p (j n) -> p j n", j=5),
                            ohb[:].unsqueeze(1).broadcast_to([128, 5, N_BLOCK]), Alu.mult)
                        inpq = sm.tile([128, 5], dt.float32, tag="inpq", name="inpq")
                        nc.vector.tensor_reduce(inpq[:], gtmp[:], AX.X, Alu.add)
                        pxt = lps.tile([5, 128], dt.float32, tag="psxt", bufs=2, name="pxt")
                        nc.tensor.matmul(pxt[:], inpq[:], ident[:], start=True, stop=True)
                        nc.vector.tensor_copy(xT[:, qs], pxt[:])
                        # ---- slot selection ----
                        pls = lps.tile([128, N_SLOT], dt.float32, tag="pslog", bufs=2,
                                       name="pls")
                        nc.tensor.matmul(pls[:], hto[0][:, qs], wst[0][:], start=True, stop=False)
                        nc.tensor.matmul(pls[:], hto[1][:, qs], wst[1][:], start=False, stop=False)
                        nc.tensor.matmul(pls[:], ones1[:, 0:128], bs_row[:], start=False, stop=True)
                        ge0 = sm.tile([128, N_SLOT], dt.float32, tag="ge0", name="ge0")
                        ge1 = sm.tile([128, N_SLOT], dt.float32, tag="ge1", name="ge1")
                        nc.vector.tensor_scalar(ge0[:], d0p[q][:], inpq[:, 0:1], None, Alu.is_gt)
                        nc.vector.tensor_scalar(ge1[:], d1p[q][:], inpq[:, 1:2], None, Alu.is_gt)
                        fe01 = sm.tile([128, N_SLOT], dt.float32, tag="fe01", name="fe01")
                        nc.vector.tensor_tensor(fe01[:], ge0[:], ge1[:], Alu.mult)
                        alv = sm.tile([128, N_SLOT], dt.float32, tag="alv", name="alv")
                        nc.vector.tensor_scalar(alv[:], cntp[q][:], 0.0, None, Alu.is_gt)
                        feas = sm.tile([128, N_SLOT], dt.float32, tag="feas", name="feas")
                        nc.vector.tensor_tensor(feas[:], fe01[:], alv[:], Alu.mult)
                        tadd = sm.tile([128, N_SLOT], dt.float32, tag="tadd", name="tadd")
                        nc.vector.tensor_scalar(tadd[:], feas[:], 2000.0, -2000.0,
                                                Alu.mult, Alu.add)
                        msk = sm.tile([128, N_SLOT], dt.float32, tag="msk", name="msk")
                        nc.vector.tensor_tensor(msk[:], pls[:], tadd[:], Alu.add)
                        mxs = sm.tile([128, 1], dt.float32, tag="mxs", name="mxs")
                        nc.vector.tensor_reduce(mxs[:], msk[:], AX.X, Alu.max)
                        idxs = sm.tile([128, 8], dt.uint32, tag="idxs", name="idxs")
                        nc.vector.max_index(idxs[:], mxs[:, 0:1].broadcast_to([128, 8]), msk[:])
                        nc.vector.tensor_copy(acts_[q][:, ds(t, 1)], idxs[:, 0:1])
                        sels = sm.tile([128, 1], dt.float32, tag="sels", name="sels")
                        nc.vector.tensor_copy(sels[:], idxs[:, 0:1])
                        ohs = sm.tile([128, N_SLOT], dt.float32, tag="ohs", name="ohs")
                        nc.vector.tensor_scalar(ohs[:], iota_f[:], sels[:, 0:1], None,
                                                Alu.is_equal)
                        nc.vector.scalar_tensor_tensor(cntp[q][:], ohs[:], -1.0, cntp[q][:],
                                                       Alu.mult, Alu.add)
                        # ---- block selection (ti) ----
                        plt = lps.tile([128, N_BLOCK], dt.float32, tag="pslog", bufs=2,
                                       name="plt")
                        nc.tensor.matmul(plt[:], hti[0][:, qs], wbt[0][:], start=True, stop=False)
                        nc.tensor.matmul(plt[:], hti[1][:, qs], wbt[1][:], start=False, stop=True)
                        mt = sm.tile([128, N_BLOCK], dt.float32, tag="mt", name="mt")
                        nc.vector.tensor_tensor(mt[:], plt[:], madd_ti[q][:], Alu.add)
                        mxt = sm.tile([128, 1], dt.float32, tag="mxt", name="mxt")
                        nc.vector.tensor_reduce(mxt[:], mt[:], AX.X, Alu.max)
                        idxt = sm.tile([128, 8], dt.uint32, tag="idxt", name="idxt")
                        nc.vector.max_index(idxt[:], mxt[:, 0:1].broadcast_to([128, 8]), mt[:])
                        nc.vector.tensor_copy(actt[q][:, ds(t, 1)], idxt[:, 0:1])
                        selt = sm.tile([128, 1], dt.float32, tag="selt", name="selt")
                        nc.vector.tensor_copy(selt[:], idxt[:, 0:1])
                        oht = sm.tile([128, N_BLOCK], dt.float32, tag="oht", name="oht")
                        nc.vector.tensor_scalar(oht[:], iota_f[:, 0:N_BLOCK], selt[:, 0:1],
                                                None, Alu.is_equal)
                        nc.vector.scalar_tensor_tensor(madd_ti[q][:], oht[:], -2000.0,
                                                       madd_ti[q][:], Alu.mult, Alu.add)

                for q in range(NQ):
                    r0 = q * 128
                    nc.sync.dma_start(act_b_d[r0:r0 + 128, :], actb[q][:])
                    nc.sync.dma_start(act_s_d[r0:r0 + 128, :], acts_[q][:])
                    nc.sync.dma_start(act_t_d[r0:r0 + 128, :], actt[q][:])

    nc.compile()
    return nc


def _prep_inputs(inputs):
    """Host-side exact preprocessing -> per-core in_maps."""
    f32 = np.float32
    to = np.ascontiguousarray(inputs["to_tensor"], dtype=f32)
    ti = np.ascontiguousarray(inputs["ti_tensor"], dtype=f32)
    ys = np.ascontiguousarray(inputs["ys_tensor"], dtype=f32)
    slot = np.ascontiguousarray(inputs["slot_info"], dtype=f32)
    W1 = np.asarray(inputs["W1"], f32); b1 = np.asarray(inputs["b1"], f32)
    W2 = np.asarray(inputs["W2"], f32); b2 = np.asarray(inputs["b2"], f32)
    W3 = np.asarray(inputs["W3"], f32); b3 = np.asarray(inputs["b3"], f32)
    W_fe2 = np.asarray(inputs["W_fe2"], f32); b_fe2 = np.asarray(inputs["b_fe2"], f32)
    Wih_to = np.asarray(inputs["Wih_to"], f32); Whh_to = np.asarray(inputs["Whh_to"], f32)
    bih_to = np.asarray(inputs["bih_to"], f32); bhh_to = np.asarray(inputs["bhh_to"], f32)
    Whh_ti = np.asarray(inputs["Whh_ti"], f32)
    bih_ti = np.asarray(inputs["bih_ti"], f32); bhh_ti = np.asarray(inputs["bhh_ti"], f32)
    Wb = np.asarray(inputs["Wb"], f32); bb = np.asarray(inputs["bb"], f32)
    Ws = np.asarray(inputs["Ws"], f32); bs = np.asarray(inputs["bs"], f32)

    def gate_bias_pack(bsum):
        cols = []
        for m in range(8):
            seg = bsum[m * 128:(m + 1) * 128]
            cols.append(seg if m in (4, 5) else seg * 0.5)
        return np.stack(cols, axis=1).astype(f32)

    shared = {
        "w1t": np.ascontiguousarray(W1.T),
        "w2t": np.ascontiguousarray(W2.T),
        "w3t": np.ascontiguousarray(W3.T),
        "b1p": np.ascontiguousarray(b1.reshape(4, 128).T),
        "b2p": np.ascontiguousarray(b2.reshape(4, 128).T),
        "b3p": np.ascontiguousarray(b3.reshape(2, 128).T),
        "w2f": np.broadcast_to(2.0 * W_fe2[0], (128, 5)).astype(f32),
        "bfe": np.full((128, 1), 2.0 * b_fe2[0], f32),
        "whht_to": np.ascontiguousarray((Whh_to * 0.5).T),
        "whht_ti": np.ascontiguousarray((Whh_ti * 0.5).T),
        "wiht_to": np.ascontiguousarray(Wih_to.T),
        "bg_to": gate_bias_pack((bih_to + bhh_to).astype(f32)),
        "bg_ti": gate_bias_pack((bih_ti + bhh_ti).astype(f32)),
        "wbt": np.ascontiguousarray((Wb * 0.5).T),
        "wst": np.ascontiguousarray((Ws * 0.5).T),
        "bs_row": bs.reshape(1, N_SLOT),
    }
    in_maps = []
    for c in range(NCORE):
        s = slice(c * BL, (c + 1) * BL)
        toc, tic, ysc, slc = to[s], ti[s], ys[s], slot[s]
        m_to = np.where(toc[:, :, 0] != -1.0, bb[None, :], bb[None, :] - 2000.0).astype(f32)
        m_ti = np.where(tic[:, :, 0] != -1.0, bb[None, :], bb[None, :] - 2000.0).astype(f32)
        in_maps.append(dict(shared,
            to_r=np.ascontiguousarray(toc.reshape(BL, -1)),
            ti_r=np.ascontiguousarray(tic.reshape(BL, -1)),
            ys_r=np.ascontiguousarray(ysc.reshape(BL, -1)),
            to_feats=np.ascontiguousarray(toc.transpose(0, 2, 1)).reshape(BL, -1),
            d0=np.ascontiguousarray(slc[:, :, 0]),
            d1=np.ascontiguousarray(slc[:, :, 1]),
            cnt=np.ascontiguousarray(slc[:, :, 4]),
            madd_to=m_to, madd_ti=m_ti))
    return in_maps


def kernel(**inputs):
    if "nc" not in _CACHE:
        _CACHE["nc"] = _build(STEPS)
    nc = _CACHE["nc"]
    from concourse.bass_utils import run_bass_kernel_spmd
    in_maps = _prep_inputs(inputs)
    results = run_bass_kernel_spmd(nc, in_maps, list(range(NCORE))).results
    actions = np.empty((B, STEPS, 3), np.int32)
    for c in range(NCORE):
        s = slice(c * BL, (c + 1) * BL)
        actions[s, :, 0] = results[c]["act_b"]
        actions[s, :, 1] = results[c]["act_s"]
        actions[s, :, 2] = results[c]["act_t"]
    state = np.concatenate([
        np.asarray(inputs["to_tensor"], np.float32),
        np.asarray(inputs["ti_tensor"], np.float32),
        np.asarray(inputs["ys_tensor"], np.float32)], axis=1)
    return state, actions
